# revision 43
# baseline (speedup 1.0000x reference)
"""Complex-valued attention (magnitude-softmax with phase reconstruction) on 8 TRN2 cores.

Sharding: core c -> (batch b = c//2, query-half qh = c%2). No collectives:
each core computes a disjoint [512, 1024] slice of the output. The query
half is selected by permuting the token axis of x^T host-side so that each
core's queries are always columns 0:512 of its shard (SPMD: one graph).

Math (per head h, scale S = 1/8):
  w = x @ Wqkv^T                         (bf16 matmuls, fp32 accum)
  z = dots^T[k, q] = <w_k-ish>           computed TRANSPOSED via stacked
      re/im contraction so softmax ends up on the partition axis
  m2 = z_re^2 + z_im^2; l = ln(m2+eps); t = 0.5*l + ln(S)  (t = ln(S*mag))
  e1 = exp(t) = S*mag;  e = exp(e1)  (softmax numerator, no max needed)
  f = exp(e1 - t) = e/(S*mag);  attnU = z * f   (= 8 * e * unit_phase)
  oh^T = sum_k wcombo^T @ attnU^T  ; normalized by 1/(8*sum_k e)
  out = oh @ Wout^T  (+bias, which is zero)
"""

import os
import sys
import numpy as np
import ml_dtypes

sys.path.insert(0, "/opt/trn_rl_repo")

from contextlib import ExitStack

import concourse.bass as bass
import concourse.tile as tile
from concourse import bacc, mybir, dve_ops
from concourse.bass_utils import run_bass_kernel_spmd
from concourse.dve_spec import Spec, Src0, Src1, sq, lower, _has_src1
from concourse.dve_uop import DveOpSpec


def _register_sqmag():
    """Custom DVE op: out = Src0^2 + Src1^2 (one instr instead of 2 ACT + 1 DVE)."""
    name = "TENSOR_SQMAG_ANT"
    if name in dve_ops._SUB_OPCODE_FOR_NAME:
        return next(o for o in dve_ops.OPS if o.name == name)
    spec = Spec(
        body=sq(Src0) + sq(Src1),
        reference=lambda in0, in1, s0, s1, imm2:
            (in0.astype(np.float32) ** 2 + in1.astype(np.float32) ** 2),
    )
    opcode = max(dve_ops._SUB_OPCODE_FOR_NAME.values()) + 1
    dve_ops._SUB_OPCODE_FOR_NAME[name] = opcode
    shas = {}
    for ver in ("v3", "v4"):
        uops = lower(spec, ver=ver)
        shas[ver] = DveOpSpec(name=name, opcode=opcode, uops=uops,
                              rd1_en=_has_src1(spec)).sha(ver)
    op = dve_ops.DveOp(name, spec, subdim=False, uops_sha=shas)
    dve_ops.OPS.append(op)
    dve_ops.CUSTOM_DVE_SPECS[name] = spec
    return op


SQMAG = _register_sqmag()


def _patch_act_tables():
    """Force exp/ln to resolve to the combined natural_log_exp_and_others set
    so the per-tile Ln->Exp alternation doesn't reload ACT tables (~2.7us each)."""
    import concourse.bacc as _bacc
    if getattr(_bacc, "_act_tables_patched", False):
        return
    orig = _bacc.get_activation_tables
    AFT = mybir.ActivationFunctionType

    def patched(arch):
        t = {k: set(v) for k, v in orig(arch).items()}
        for name, fns in t.items():
            if name != "natural_log_exp_and_others":
                fns.discard(AFT.Exp)
                fns.discard(AFT.Ln)
        return t

    _bacc.get_activation_tables = patched
    _bacc._act_tables_patched = True


_patch_act_tables()

B, N, D, H, DH = 4, 1024, 1024, 16, 64
E = H * DH          # 1024
NQ = 512            # queries per core
KT = 8              # key tiles of 128
DT = 8              # d (contraction) tiles of 128
ET = 8              # e tiles of 128 (2 heads each)
SCALE = DH ** -0.5  # 0.125
LN_S = float(np.log(SCALE))
EPS = 1e-20

FP32 = mybir.dt.float32
BF16 = mybir.dt.bfloat16
AF = mybir.ActivationFunctionType
ALU = mybir.AluOpType

_CACHE = {}


def build_graph():
    nc = bacc.Bacc("TRN2", target_bir_lowering=False, debug=False,
                   enable_asserts=False, num_devices=8)

    xTr_d = nc.declare_dram_parameter("xTr", [D, N], BF16, isOutput=False)
    xTi_d = nc.declare_dram_parameter("xTi", [D, N], BF16, isOutput=False)
    wqr_d = nc.declare_dram_parameter("wqTr", [D, E], BF16, isOutput=False)   # Wqkv_re.T
    wqi_d = nc.declare_dram_parameter("wqTi", [D, E], BF16, isOutput=False)   # Wqkv_im.T
    wqin_d = nc.declare_dram_parameter("wqTin", [D, E], BF16, isOutput=False)  # -Wqkv_im.T
    wo_d = nc.declare_dram_parameter("woS", [2 * E, D], BF16, isOutput=False)  # [Wout_re.T; Wout_im.T]
    our_d = nc.declare_dram_parameter("out_re", [NQ, D], FP32, isOutput=True)
    oui_d = nc.declare_dram_parameter("out_im", [NQ, D], FP32, isOutput=True)

    with tile.TileContext(nc) as tc, ExitStack() as ctx:
        const_pool = ctx.enter_context(tc.tile_pool(name="const", bufs=1))
        xpool = ctx.enter_context(tc.tile_pool(name="x", bufs=1))
        wqpool = ctx.enter_context(tc.tile_pool(name="wq", bufs=1))
        apool = ctx.enter_context(tc.tile_pool(name="A", bufs=1))
        bpool = ctx.enter_context(tc.tile_pool(name="B", bufs=2))
        wcpool = ctx.enter_context(tc.tile_pool(name="wc", bufs=2))
        ohpool = ctx.enter_context(tc.tile_pool(name="oh", bufs=1))
        wopool = ctx.enter_context(tc.tile_pool(name="wo", bufs=1))
        epool = ctx.enter_context(tc.tile_pool(name="elem", bufs=2))
        spool = ctx.enter_context(tc.tile_pool(name="sm", bufs=2))
        opool = ctx.enter_context(tc.tile_pool(name="ostage", bufs=1))
        psd = ctx.enter_context(tc.tile_pool(name="psd", bufs=3, space="PSUM"))
        psoh = ctx.enter_context(tc.tile_pool(name="psoh", bufs=1, space="PSUM"))
        psS = ctx.enter_context(tc.tile_pool(name="psS", bufs=1, space="PSUM"))

        # ---- constants ----
        ones_bf = const_pool.tile([128, 1], BF16, tag="ones")
        nc.gpsimd.memset(ones_bf[:], 1.0)
        eps_t = const_pool.tile([128, 1], FP32, tag="eps")
        nc.gpsimd.memset(eps_t[:], EPS)
        lnS_t = const_pool.tile([128, 1], FP32, tag="lnS")
        nc.gpsimd.memset(lnS_t[:], LN_S)
        ones8 = const_pool.tile([1, 128], FP32, tag="ones8")
        nc.gpsimd.memset(ones8[:], SCALE)

        # ---- resident loads ----
        # x^T as [128, DT, N] (partition = d within tile)
        xr = xpool.tile([128, DT, N], BF16, tag="xr")
        xi = xpool.tile([128, DT, N], BF16, tag="xi")
        nc.sync.dma_start(out=xr[:], in_=xTr_d.ap().rearrange("(t p) n -> p t n", p=128))
        nc.sync.dma_start(out=xi[:], in_=xTi_d.ap().rearrange("(t p) n -> p t n", p=128))
        # A stacks: per head [128 (wr 0:64 | wi 64:128), N]
        A = [apool.tile([128, N], BF16, tag=f"A{h}", name=f"A{h}") for h in range(H)]

        # ---- stage 1: w^T = Wqkv~ @ x^T, evicted to per-head stacks ----
        for et in range(ET):
            wslab_r = wqpool.tile([128, DT, 128], BF16, tag="wslab_r")
            wslab_i = wqpool.tile([128, DT, 128], BF16, tag="wslab_i")
            wslab_in = wqpool.tile([128, DT, 128], BF16, tag="wslab_in")
            esl = slice(et * 128, (et + 1) * 128)
            nc.sync.dma_start(out=wslab_r[:], in_=wqr_d.ap()[:, esl].rearrange("(t p) n -> p t n", p=128))
            nc.sync.dma_start(out=wslab_i[:], in_=wqi_d.ap()[:, esl].rearrange("(t p) n -> p t n", p=128))
            nc.sync.dma_start(out=wslab_in[:], in_=wqin_d.ap()[:, esl].rearrange("(t p) n -> p t n", p=128))
            ps_re = psd.tile([128, 2, 512], FP32, tag="dots")
            ps_im = psd.tile([128, 2, 512], FP32, tag="dots")
            for nch in range(2):
                nsl = slice(nch * 512, (nch + 1) * 512)
                for dt_ in range(DT):
                    first, last = dt_ == 0, dt_ == DT - 1
                    # w_re += Wr^T x_r ; w_re += (-Wi^T) x_i
                    nc.tensor.matmul(ps_re[:, nch, :], wslab_r[:, dt_, :], xr[:, dt_, nsl],
                                     start=first, stop=False)
                    nc.tensor.matmul(ps_re[:, nch, :], wslab_in[:, dt_, :], xi[:, dt_, nsl],
                                     start=False, stop=last)
                    # w_im += Wi^T x_r ; w_im += Wr^T x_i
                    nc.tensor.matmul(ps_im[:, nch, :], wslab_i[:, dt_, :], xr[:, dt_, nsl],
                                     start=first, stop=False)
                    nc.tensor.matmul(ps_im[:, nch, :], wslab_r[:, dt_, :], xi[:, dt_, nsl],
                                     start=False, stop=last)
            # evict into head stacks (cast to bf16)
            h0, h1 = 2 * et, 2 * et + 1
            Ar = A[h0].rearrange("p (c n) -> p c n", c=2)
            Ai = A[h1].rearrange("p (c n) -> p c n", c=2)
            nc.scalar.copy(Ar[0:64, :, :], ps_re[0:64, :, :])
            nc.scalar.copy(Ai[0:64, :, :], ps_re[64:128, :, :])
            nc.scalar.copy(Ar[64:128, :, :], ps_im[0:64, :, :])
            nc.scalar.copy(Ai[64:128, :, :], ps_im[64:128, :, :])

        # ---- per-head attention ----
        # oh^T stacks for stage 4: [128, ET, NQ] bf16
        ohr = ohpool.tile([128, ET, NQ], BF16, tag="ohr")
        ohi = ohpool.tile([128, ET, NQ], BF16, tag="ohi")
        ohin = ohpool.tile([128, ET, NQ], BF16, tag="ohin")

        for h in range(H):
            Ah = A[h]
            # B_h = [-wi; wr]
            Bh = bpool.tile([128, N], BF16, tag="B")
            nc.vector.tensor_scalar_mul(Bh[0:64, :], Ah[64:128, :], -1.0)
            nc.vector.tensor_copy(Bh[64:128, :], Ah[0:64, :])

            # wcombo1 = w_nat [k, (wr|wi)]; wcombo2 = [-wi|wr] per k-tile
            wc1 = wcpool.tile([128, KT, 128], BF16, tag="wc1")
            wc2 = wcpool.tile([128, KT, 128], BF16, tag="wc2")
            for kt in range(KT):
                ksl = slice(kt * 128, (kt + 1) * 128)
                nc.sync.dma_start(wc1[:, kt, :], Ah[:, ksl], transpose=True)
                nc.sync.dma_start(wc2[:, kt, :], Bh[:, ksl], transpose=True)

            ps_oh = psoh.tile([128, NQ], FP32, tag="oh")
            ps_s = psS.tile([1, NQ], FP32, tag="S")
            zz_pairs = {}

            for kq in range(KT // 4):
              m2q = epool.tile([128, 4, NQ], FP32, tag="m2q", bufs=2)
              llq = epool.tile([128, 4, NQ], FP32, tag="llq", bufs=1)
              e1q = epool.tile([128, 4, NQ], FP32, tag="e1q", bufs=2)
              eeq = epool.tile([128, 4, NQ], BF16, tag="eeq", bufs=2)
              for kp in (2 * kq, 2 * kq + 1):
                zre = psd.tile([128, 2, NQ], FP32, tag="dots")
                zim = psd.tile([128, 2, NQ], FP32, tag="dots")
                # dots^T[k, q]: re = [wr;wi]_k . [wr;wi]_q ; im = [-wi;wr]_k . [wr;wi]_q
                for i in range(2):
                    ksl = slice((2 * kp + i) * 128, (2 * kp + i + 1) * 128)
                    nc.tensor.matmul(zre[:, i, :], Ah[:, ksl], Ah[:, 0:NQ],
                                     start=True, stop=True)
                    nc.tensor.matmul(zim[:, i, :], Bh[:, ksl], Ah[:, 0:NQ],
                                     start=True, stop=True)

                # evict both dots tensors as bf16: PSUM recycles fast and the
                # tail multiplies run in DVE 2x mode
                zreS = epool.tile([128, 2, NQ], BF16, tag="zreS", bufs=3)
                nc.scalar.copy(zreS[:], zre[:])
                zimS = epool.tile([128, 2, NQ], BF16, tag="zimS", bufs=3)
                if kp % 2 == 0:
                    nc.scalar.copy(zimS[:], zim[:])
                else:
                    nc.vector.tensor_copy(zimS[:], zim[:])
                zz_pairs[kp % 2] = (zreS, zimS)
                hh = kp % 2  # which half of the quad
                hsl = slice(2 * hh, 2 * hh + 2)
                nc.vector._custom_dve(SQMAG, out=m2q[:, hsl, :], in0=zreS[:], in1=zimS[:])
                if hh == 1:
                    nc.scalar.activation(llq[:], m2q[:], AF.Ln, bias=eps_t[:])
                    # e1 = S*mag = exp(0.5*ln(m2) + ln(S)) -- affine fused
                    nc.scalar.activation(e1q[:], llq[:], AF.Exp, scale=0.5, bias=lnS_t[:])
                    nc.scalar.activation(eeq[:], e1q[:], AF.Exp)
                for back in ((0, 1) if hh == 1 else ()):
                    kpb = 2 * kq + back
                    bsl = slice(2 * back, 2 * back + 2)
                    rm = epool.tile([128, 2, NQ], FP32, tag="rm", bufs=1)
                    nc.vector.reciprocal_approx_fast(out=rm[:], in_=e1q[:, bsl, :])
                    ff = epool.tile([128, 2, NQ], BF16, tag="ff", bufs=2)
                    nc.vector.tensor_mul(ff[:], eeq[:, bsl, :], rm[:])
                    are = epool.tile([128, 2, NQ], BF16, tag="are", bufs=2)
                    aim = epool.tile([128, 2, NQ], BF16, tag="aim", bufs=2)
                    zreSb, zimSb = zz_pairs[kpb % 2]
                    nc.vector.tensor_mul(are[:], zreSb[:], ff[:])
                    nc.vector.tensor_mul(aim[:], zimSb[:], ff[:])
                    for i in range(2):
                        kt = 2 * kpb + i
                        first, last = kt == 0, kt == KT - 1
                        nc.tensor.matmul(ps_oh[:], wc1[:, kt, :], are[:, i, :],
                                         start=first, stop=False)
                        nc.tensor.matmul(ps_oh[:], wc2[:, kt, :], aim[:, i, :],
                                         start=False, stop=last)
                        nc.tensor.matmul(ps_s[:], ones_bf[:], eeq[:, bsl, :][:, i, :],
                                         start=first, stop=last)

            # evict raw oh^T immediately (releases psoh for the next head)
            ohraw = spool.tile([128, NQ], FP32, tag="ohraw", bufs=1)
            nc.scalar.copy(ohraw[:], ps_oh[:])
            # rs = 1/S (psS released); broadcast SCALE/S via ones-matmul into PSUM
            rs = spool.tile([1, NQ], FP32, tag="rs", bufs=1)
            nc.vector.reciprocal_approx_fast(out=rs[:], in_=ps_s[:])
            bb = psd.tile([128, 2, NQ], FP32, tag="dots")
            nc.tensor.matmul(bb[:, 0, :], ones8[:], rs[:], start=True, stop=True)

            et2, half = h // 2, (h % 2) * 64
            hs = slice(half, half + 64)
            nc.vector.tensor_mul(ohr[hs, et2, :], ohraw[0:64, :], bb[0:64, 0, :])
            nc.vector.tensor_mul(ohi[hs, et2, :], ohraw[64:128, :], bb[64:128, 0, :])
            nc.vector.tensor_scalar_mul(ohin[hs, et2, :], ohi[hs, et2, :], -1.0)

        # ---- stage 4: out = oh @ Wout^T ----
        for dc in range(2):
            dsl = slice(dc * 512, (dc + 1) * 512)
            wos = wopool.tile([128, 16, 512], BF16, tag="wos")
            nc.sync.dma_start(out=wos[:], in_=wo_d.ap()[:, dsl].rearrange("(t p) n -> p t n", p=128))
            for qt in range(4):
                qsl = slice(qt * 128, (qt + 1) * 128)
                po = psd.tile([128, 2, 512], FP32, tag="dots")
                for et in range(ET):
                    first = et == 0
                    nc.tensor.matmul(po[:, 0, :], ohr[:, et, qsl], wos[:, et, :],
                                     start=first, stop=False)
                    nc.tensor.matmul(po[:, 0, :], ohin[:, et, qsl], wos[:, ET + et, :],
                                     start=False, stop=(et == ET - 1))
                    nc.tensor.matmul(po[:, 1, :], ohi[:, et, qsl], wos[:, et, :],
                                     start=first, stop=False)
                    nc.tensor.matmul(po[:, 1, :], ohr[:, et, qsl], wos[:, ET + et, :],
                                     start=False, stop=(et == ET - 1))
                o_st = opool.tile([128, 2, 512], FP32, tag="ost", bufs=1)
                nc.scalar.copy(o_st[:], po[:])
                nc.sync.dma_start(out=our_d.ap()[qsl, dsl], in_=o_st[:, 0, :])
                nc.sync.dma_start(out=oui_d.ap()[qsl, dsl], in_=o_st[:, 1, :])

    nc.compile()
    return nc


def _to_bf16(a):
    return np.asarray(a, dtype=np.float32).astype(ml_dtypes.bfloat16)


def make_in_maps(x_re, x_im, wqkv_re, wqkv_im, wout_re, wout_im, bout_re, bout_im):
    x_re = np.asarray(x_re, np.float32)
    x_im = np.asarray(x_im, np.float32)
    wq_r = _to_bf16(np.asarray(wqkv_re, np.float32).T)
    wq_i = _to_bf16(np.asarray(wqkv_im, np.float32).T)
    wq_in = _to_bf16(-np.asarray(wqkv_im, np.float32).T)
    wo_s = _to_bf16(np.concatenate([np.asarray(wout_re, np.float32).T,
                                    np.asarray(wout_im, np.float32).T], axis=0))

    in_maps = []
    for c in range(8):
        b, qh = c // 2, c % 2
        xtr = x_re[b].T
        xti = x_im[b].T
        if qh == 1:
            xtr = np.concatenate([xtr[:, NQ:], xtr[:, :NQ]], axis=1)
            xti = np.concatenate([xti[:, NQ:], xti[:, :NQ]], axis=1)
        in_maps.append({
            "xTr": _to_bf16(np.ascontiguousarray(xtr)),
            "xTi": _to_bf16(np.ascontiguousarray(xti)),
            "wqTr": wq_r, "wqTi": wq_i, "wqTin": wq_in, "woS": wo_s,
        })
    return in_maps


def assemble_output(res, bout_re, bout_im):
    out = np.zeros((B, N, D), np.complex64)
    for c in range(8):
        b, qh = c // 2, c % 2
        rows = slice(0, NQ) if qh == 0 else slice(NQ, N)
        out[b, rows, :] = res[c]["out_re"] + 1j * res[c]["out_im"]

    # bout is zero in this problem; add anyway for faithfulness
    out += (np.asarray(bout_re, np.float32) + 1j * np.asarray(bout_im, np.float32))[None, None, :]
    return out


def kernel(**inputs):
    if "nc" not in _CACHE:
        _CACHE["nc"] = build_graph()
    nc = _CACHE["nc"]
    in_maps = make_in_maps(**inputs)
    res = run_bass_kernel_spmd(nc, in_maps, core_ids=list(range(8))).results
    return assemble_output(res, inputs["bout_re"], inputs["bout_im"])


# revision 44
# speedup vs baseline: 1.2610x; 1.2610x over previous
"""Complex-valued attention (magnitude-softmax with phase reconstruction) on 8 TRN2 cores.

Sharding: core c -> (batch b = c//2, query-half qh = c%2). No collectives:
each core computes a disjoint [512, 1024] slice of the output. The query
half is selected by permuting the token axis of x^T host-side so that each
core's queries are always columns 0:512 of its shard (SPMD: one graph).

Math (per head h, scale S = 1/8):
  w = x @ Wqkv^T                         (bf16 matmuls, fp32 accum)
  z = dots^T[k, q] = <w_k-ish>           computed TRANSPOSED via stacked
      re/im contraction so softmax ends up on the partition axis
  m2 = z_re^2 + z_im^2; l = ln(m2+eps); t = 0.5*l + ln(S)  (t = ln(S*mag))
  e1 = exp(t) = S*mag;  e = exp(e1)  (softmax numerator, no max needed)
  f = exp(e1 - t) = e/(S*mag);  attnU = z * f   (= 8 * e * unit_phase)
  oh^T = sum_k wcombo^T @ attnU^T  ; normalized by 1/(8*sum_k e)
  out = oh @ Wout^T  (+bias, which is zero)
"""

import os
import sys
import numpy as np
import ml_dtypes

sys.path.insert(0, "/opt/trn_rl_repo")

from contextlib import ExitStack

import concourse.bass as bass
import concourse.tile as tile
from concourse import bacc, mybir, dve_ops
from concourse.bass_utils import run_bass_kernel_spmd
from concourse.dve_spec import Spec, Src0, Src1, sq, lower, _has_src1
from concourse.dve_uop import DveOpSpec


def _register_sqmag():
    """Custom DVE op: out = Src0^2 + Src1^2 (one instr instead of 2 ACT + 1 DVE)."""
    name = "TENSOR_SQMAG_ANT"
    if name in dve_ops._SUB_OPCODE_FOR_NAME:
        return next(o for o in dve_ops.OPS if o.name == name)
    spec = Spec(
        body=sq(Src0) + sq(Src1),
        reference=lambda in0, in1, s0, s1, imm2:
            (in0.astype(np.float32) ** 2 + in1.astype(np.float32) ** 2),
    )
    opcode = max(dve_ops._SUB_OPCODE_FOR_NAME.values()) + 1
    dve_ops._SUB_OPCODE_FOR_NAME[name] = opcode
    shas = {}
    for ver in ("v3", "v4"):
        uops = lower(spec, ver=ver)
        shas[ver] = DveOpSpec(name=name, opcode=opcode, uops=uops,
                              rd1_en=_has_src1(spec)).sha(ver)
    op = dve_ops.DveOp(name, spec, subdim=False, uops_sha=shas)
    dve_ops.OPS.append(op)
    dve_ops.CUSTOM_DVE_SPECS[name] = spec
    return op


SQMAG = _register_sqmag()


def _patch_act_tables():
    """Force exp/ln to resolve to the combined natural_log_exp_and_others set
    so the per-tile Ln->Exp alternation doesn't reload ACT tables (~2.7us each)."""
    import concourse.bacc as _bacc
    if getattr(_bacc, "_act_tables_patched", False):
        return
    orig = _bacc.get_activation_tables
    AFT = mybir.ActivationFunctionType

    def patched(arch):
        t = {k: set(v) for k, v in orig(arch).items()}
        for name, fns in t.items():
            if name != "natural_log_exp_and_others":
                fns.discard(AFT.Exp)
                fns.discard(AFT.Ln)
        return t

    _bacc.get_activation_tables = patched
    _bacc._act_tables_patched = True


_patch_act_tables()

B, N, D, H, DH = 4, 1024, 1024, 16, 64
E = H * DH          # 1024
NQ = 512            # queries per core
KT = 8              # key tiles of 128
DT = 8              # d (contraction) tiles of 128
ET = 8              # e tiles of 128 (2 heads each)
SCALE = DH ** -0.5  # 0.125
LN_S = float(np.log(SCALE))
EPS = 1e-20

FP32 = mybir.dt.float32
BF16 = mybir.dt.bfloat16
AF = mybir.ActivationFunctionType
ALU = mybir.AluOpType

_CACHE = {}


def build_graph():
    nc = bacc.Bacc("TRN2", target_bir_lowering=False, debug=False,
                   enable_asserts=False, num_devices=8)

    xTr_d = nc.declare_dram_parameter("xTr", [D, N], BF16, isOutput=False)
    xTi_d = nc.declare_dram_parameter("xTi", [D, N], BF16, isOutput=False)
    wqr_d = nc.declare_dram_parameter("wqTr", [D, E], BF16, isOutput=False)   # Wqkv_re.T
    wqi_d = nc.declare_dram_parameter("wqTi", [D, E], BF16, isOutput=False)   # Wqkv_im.T
    wqin_d = nc.declare_dram_parameter("wqTin", [D, E], BF16, isOutput=False)  # -Wqkv_im.T
    wo_d = nc.declare_dram_parameter("woS", [2 * E, D], BF16, isOutput=False)  # [Wout_re.T; Wout_im.T]
    our_d = nc.declare_dram_parameter("out_re", [NQ, D], FP32, isOutput=True)
    oui_d = nc.declare_dram_parameter("out_im", [NQ, D], FP32, isOutput=True)

    with tile.TileContext(nc) as tc, ExitStack() as ctx:
        const_pool = ctx.enter_context(tc.tile_pool(name="const", bufs=1))
        xpool = ctx.enter_context(tc.tile_pool(name="x", bufs=1))
        wqpool = ctx.enter_context(tc.tile_pool(name="wq", bufs=1))
        apool = ctx.enter_context(tc.tile_pool(name="A", bufs=1))
        bpool = ctx.enter_context(tc.tile_pool(name="B", bufs=2))
        wcpool = ctx.enter_context(tc.tile_pool(name="wc", bufs=2))
        ohpool = ctx.enter_context(tc.tile_pool(name="oh", bufs=1))
        wopool = ctx.enter_context(tc.tile_pool(name="wo", bufs=1))
        epool = ctx.enter_context(tc.tile_pool(name="elem", bufs=2))
        spool = ctx.enter_context(tc.tile_pool(name="sm", bufs=2))
        opool = ctx.enter_context(tc.tile_pool(name="ostage", bufs=1))
        psd = ctx.enter_context(tc.tile_pool(name="psd", bufs=3, space="PSUM"))
        psoh = ctx.enter_context(tc.tile_pool(name="psoh", bufs=1, space="PSUM"))
        psS = ctx.enter_context(tc.tile_pool(name="psS", bufs=1, space="PSUM"))

        # ---- constants ----
        ones_bf = const_pool.tile([128, 1], BF16, tag="ones")
        nc.gpsimd.memset(ones_bf[:], 1.0)
        eps_t = const_pool.tile([128, 1], FP32, tag="eps")
        nc.gpsimd.memset(eps_t[:], EPS)
        lnS_t = const_pool.tile([128, 1], FP32, tag="lnS")
        nc.gpsimd.memset(lnS_t[:], LN_S)
        ones8 = const_pool.tile([1, 128], FP32, tag="ones8")
        nc.gpsimd.memset(ones8[:], SCALE)

        # ---- resident loads ----
        # x^T as [128, DT, N] (partition = d within tile)
        xr = xpool.tile([128, DT, N], BF16, tag="xr")
        xi = xpool.tile([128, DT, N], BF16, tag="xi")
        nc.sync.dma_start(out=xr[:], in_=xTr_d.ap().rearrange("(t p) n -> p t n", p=128))
        nc.sync.dma_start(out=xi[:], in_=xTi_d.ap().rearrange("(t p) n -> p t n", p=128))
        # A stacks: per head [128 (wr 0:64 | wi 64:128), N]
        A = [apool.tile([128, N], BF16, tag=f"A{h}", name=f"A{h}") for h in range(H)]

        # ---- stage 1: w^T = Wqkv~ @ x^T, evicted to per-head stacks ----
        for et in range(ET):
            wslab_r = wqpool.tile([128, DT, 128], BF16, tag="wslab_r")
            wslab_i = wqpool.tile([128, DT, 128], BF16, tag="wslab_i")
            wslab_in = wqpool.tile([128, DT, 128], BF16, tag="wslab_in")
            esl = slice(et * 128, (et + 1) * 128)
            nc.sync.dma_start(out=wslab_r[:], in_=wqr_d.ap()[:, esl].rearrange("(t p) n -> p t n", p=128))
            nc.sync.dma_start(out=wslab_i[:], in_=wqi_d.ap()[:, esl].rearrange("(t p) n -> p t n", p=128))
            nc.sync.dma_start(out=wslab_in[:], in_=wqin_d.ap()[:, esl].rearrange("(t p) n -> p t n", p=128))
            ps_re = psd.tile([128, 2, 512], FP32, tag="dots")
            ps_im = psd.tile([128, 2, 512], FP32, tag="dots")
            for nch in range(2):
                nsl = slice(nch * 512, (nch + 1) * 512)
                for dt_ in range(DT):
                    first, last = dt_ == 0, dt_ == DT - 1
                    # w_re += Wr^T x_r ; w_re += (-Wi^T) x_i
                    nc.tensor.matmul(ps_re[:, nch, :], wslab_r[:, dt_, :], xr[:, dt_, nsl],
                                     start=first, stop=False)
                    nc.tensor.matmul(ps_re[:, nch, :], wslab_in[:, dt_, :], xi[:, dt_, nsl],
                                     start=False, stop=last)
                    # w_im += Wi^T x_r ; w_im += Wr^T x_i
                    nc.tensor.matmul(ps_im[:, nch, :], wslab_i[:, dt_, :], xr[:, dt_, nsl],
                                     start=first, stop=False)
                    nc.tensor.matmul(ps_im[:, nch, :], wslab_r[:, dt_, :], xi[:, dt_, nsl],
                                     start=False, stop=last)
            # evict into head stacks (cast to bf16)
            h0, h1 = 2 * et, 2 * et + 1
            Ar = A[h0].rearrange("p (c n) -> p c n", c=2)
            Ai = A[h1].rearrange("p (c n) -> p c n", c=2)
            nc.scalar.copy(Ar[0:64, :, :], ps_re[0:64, :, :])
            nc.scalar.copy(Ai[0:64, :, :], ps_re[64:128, :, :])
            nc.scalar.copy(Ar[64:128, :, :], ps_im[0:64, :, :])
            nc.scalar.copy(Ai[64:128, :, :], ps_im[64:128, :, :])

        # ---- per-head attention ----
        # oh^T stacks for stage 4: [128, ET, NQ] bf16
        ohr = ohpool.tile([128, ET, NQ], BF16, tag="ohr")
        ohi = ohpool.tile([128, ET, NQ], BF16, tag="ohi")
        ohin = ohpool.tile([128, ET, NQ], BF16, tag="ohin")

        for h in range(H):
            Ah = A[h]
            # B_h = [-wi; wr]
            Bh = bpool.tile([128, N], BF16, tag="B")
            nc.vector.tensor_scalar_mul(Bh[0:64, :], Ah[64:128, :], -1.0)
            nc.vector.tensor_copy(Bh[64:128, :], Ah[0:64, :])

            # wcombo1 = w_nat [k, (wr|wi)]; wcombo2 = [-wi|wr] per k-tile
            wc1 = wcpool.tile([128, KT, 128], BF16, tag="wc1")
            wc2 = wcpool.tile([128, KT, 128], BF16, tag="wc2")
            for kt in range(KT):
                ksl = slice(kt * 128, (kt + 1) * 128)
                nc.sync.dma_start(wc1[:, kt, :], Ah[:, ksl], transpose=True)
                nc.sync.dma_start(wc2[:, kt, :], Bh[:, ksl], transpose=True)

            ps_oh = psoh.tile([128, NQ], FP32, tag="oh")
            ps_s = psS.tile([1, NQ], FP32, tag="S")

            for kp in range(KT // 2):
                zre = psd.tile([128, 2, NQ], FP32, tag="dots")
                zim = psd.tile([128, 2, NQ], FP32, tag="dots")
                # dots^T[k, q]: re = [wr;wi]_k . [wr;wi]_q ; im = [-wi;wr]_k . [wr;wi]_q
                for i in range(2):
                    ksl = slice((2 * kp + i) * 128, (2 * kp + i + 1) * 128)
                    nc.tensor.matmul(zre[:, i, :], Ah[:, ksl], Ah[:, 0:NQ],
                                     start=True, stop=True)
                    nc.tensor.matmul(zim[:, i, :], Bh[:, ksl], Ah[:, 0:NQ],
                                     start=True, stop=True)

                # evict both dots tensors as bf16: PSUM recycles fast and the
                # tail multiplies run in DVE 2x mode
                zreS = epool.tile([128, 2, NQ], BF16, tag="zreS", bufs=3)
                nc.scalar.copy(zreS[:], zre[:])
                zimS = epool.tile([128, 2, NQ], BF16, tag="zimS", bufs=3)
                if kp % 2 == 0:
                    nc.scalar.copy(zimS[:], zim[:])
                else:
                    nc.vector.tensor_copy(zimS[:], zim[:])
                m2 = epool.tile([128, 2, NQ], FP32, tag="m2", bufs=3)
                nc.vector._custom_dve(SQMAG, out=m2[:], in0=zreS[:], in1=zimS[:])
                ll = epool.tile([128, 2, NQ], FP32, tag="ll", bufs=3)
                nc.scalar.activation(ll[:], m2[:], AF.Ln, bias=eps_t[:])
                # e1 = S*mag = exp(0.5*ln(m2) + ln(S)) -- affine fused into the ACT op
                e1 = epool.tile([128, 2, NQ], FP32, tag="e1", bufs=2)
                nc.scalar.activation(e1[:], ll[:], AF.Exp, scale=0.5, bias=lnS_t[:])
                ee = epool.tile([128, 2, NQ], BF16, tag="ee", bufs=4)
                nc.scalar.activation(ee[:], e1[:], AF.Exp)
                rm = epool.tile([128, 2, NQ], FP32, tag="rm", bufs=2)
                nc.vector.reciprocal_approx_fast(out=rm[:], in_=e1[:])
                ff = epool.tile([128, 2, NQ], BF16, tag="ff", bufs=3)
                nc.vector.tensor_mul(ff[:], ee[:], rm[:])
                are = epool.tile([128, 2, NQ], BF16, tag="are", bufs=2)
                aim = epool.tile([128, 2, NQ], BF16, tag="aim", bufs=2)
                nc.vector.tensor_mul(are[:], zreS[:], ff[:])
                nc.vector.tensor_mul(aim[:], zimS[:], ff[:])

                for i in range(2):
                    kt = 2 * kp + i
                    first, last = kt == 0, kt == KT - 1
                    nc.tensor.matmul(ps_oh[:], wc1[:, kt, :], are[:, i, :],
                                     start=first, stop=False)
                    nc.tensor.matmul(ps_oh[:], wc2[:, kt, :], aim[:, i, :],
                                     start=False, stop=last)
                    nc.tensor.matmul(ps_s[:], ones_bf[:], ee[:, i, :],
                                     start=first, stop=last)

            # evict raw oh^T immediately (releases psoh for the next head)
            ohraw = spool.tile([128, NQ], FP32, tag="ohraw", bufs=2)
            nc.scalar.copy(ohraw[:], ps_oh[:])
            # rs = 1/S (psS released); broadcast SCALE/S via ones-matmul into PSUM
            rs = spool.tile([1, NQ], FP32, tag="rs", bufs=1)
            nc.vector.reciprocal_approx_fast(out=rs[:], in_=ps_s[:])
            bb = psd.tile([128, 2, NQ], FP32, tag="dots")
            nc.tensor.matmul(bb[:, 0, :], ones8[:], rs[:], start=True, stop=True)

            et2, half = h // 2, (h % 2) * 64
            hs = slice(half, half + 64)
            nc.vector.tensor_mul(ohr[hs, et2, :], ohraw[0:64, :], bb[0:64, 0, :])
            nc.vector.tensor_mul(ohi[hs, et2, :], ohraw[64:128, :], bb[64:128, 0, :])
            nc.vector.tensor_scalar_mul(ohin[hs, et2, :], ohi[hs, et2, :], -1.0)

        # ---- stage 4: out = oh @ Wout^T ----
        for dc in range(2):
            dsl = slice(dc * 512, (dc + 1) * 512)
            wos = wopool.tile([128, 16, 512], BF16, tag="wos")
            nc.sync.dma_start(out=wos[:], in_=wo_d.ap()[:, dsl].rearrange("(t p) n -> p t n", p=128))
            for qt in range(4):
                qsl = slice(qt * 128, (qt + 1) * 128)
                po = psd.tile([128, 2, 512], FP32, tag="dots")
                for et in range(ET):
                    first = et == 0
                    nc.tensor.matmul(po[:, 0, :], ohr[:, et, qsl], wos[:, et, :],
                                     start=first, stop=False)
                    nc.tensor.matmul(po[:, 0, :], ohin[:, et, qsl], wos[:, ET + et, :],
                                     start=False, stop=(et == ET - 1))
                    nc.tensor.matmul(po[:, 1, :], ohi[:, et, qsl], wos[:, et, :],
                                     start=first, stop=False)
                    nc.tensor.matmul(po[:, 1, :], ohr[:, et, qsl], wos[:, ET + et, :],
                                     start=False, stop=(et == ET - 1))
                o_st = opool.tile([128, 2, 512], FP32, tag="ost", bufs=1)
                nc.scalar.copy(o_st[:], po[:])
                nc.sync.dma_start(out=our_d.ap()[qsl, dsl], in_=o_st[:, 0, :])
                nc.sync.dma_start(out=oui_d.ap()[qsl, dsl], in_=o_st[:, 1, :])

    nc.compile()
    return nc


def _to_bf16(a):
    return np.asarray(a, dtype=np.float32).astype(ml_dtypes.bfloat16)


def make_in_maps(x_re, x_im, wqkv_re, wqkv_im, wout_re, wout_im, bout_re, bout_im):
    x_re = np.asarray(x_re, np.float32)
    x_im = np.asarray(x_im, np.float32)
    wq_r = _to_bf16(np.asarray(wqkv_re, np.float32).T)
    wq_i = _to_bf16(np.asarray(wqkv_im, np.float32).T)
    wq_in = _to_bf16(-np.asarray(wqkv_im, np.float32).T)
    wo_s = _to_bf16(np.concatenate([np.asarray(wout_re, np.float32).T,
                                    np.asarray(wout_im, np.float32).T], axis=0))

    in_maps = []
    for c in range(8):
        b, qh = c // 2, c % 2
        xtr = x_re[b].T
        xti = x_im[b].T
        if qh == 1:
            xtr = np.concatenate([xtr[:, NQ:], xtr[:, :NQ]], axis=1)
            xti = np.concatenate([xti[:, NQ:], xti[:, :NQ]], axis=1)
        in_maps.append({
            "xTr": _to_bf16(np.ascontiguousarray(xtr)),
            "xTi": _to_bf16(np.ascontiguousarray(xti)),
            "wqTr": wq_r, "wqTi": wq_i, "wqTin": wq_in, "woS": wo_s,
        })
    return in_maps


def assemble_output(res, bout_re, bout_im):
    out = np.zeros((B, N, D), np.complex64)
    for c in range(8):
        b, qh = c // 2, c % 2
        rows = slice(0, NQ) if qh == 0 else slice(NQ, N)
        out[b, rows, :] = res[c]["out_re"] + 1j * res[c]["out_im"]

    # bout is zero in this problem; add anyway for faithfulness
    out += (np.asarray(bout_re, np.float32) + 1j * np.asarray(bout_im, np.float32))[None, None, :]
    return out


def kernel(**inputs):
    if "nc" not in _CACHE:
        _CACHE["nc"] = build_graph()
    nc = _CACHE["nc"]
    in_maps = make_in_maps(**inputs)
    res = run_bass_kernel_spmd(nc, in_maps, core_ids=list(range(8))).results
    return assemble_output(res, inputs["bout_re"], inputs["bout_im"])


# revision 45
# speedup vs baseline: 1.2982x; 1.0295x over previous
"""Complex-valued attention (magnitude-softmax with phase reconstruction) on 8 TRN2 cores.

Sharding: core c -> (batch b = c//2, query-half qh = c%2). No collectives:
each core computes a disjoint [512, 1024] slice of the output. The query
half is selected by permuting the token axis of x^T host-side so that each
core's queries are always columns 0:512 of its shard (SPMD: one graph).

Math (per head h, scale S = 1/8):
  w = x @ Wqkv^T                         (bf16 matmuls, fp32 accum)
  z = dots^T[k, q] = <w_k-ish>           computed TRANSPOSED via stacked
      re/im contraction so softmax ends up on the partition axis
  m2 = z_re^2 + z_im^2; l = ln(m2+eps); t = 0.5*l + ln(S)  (t = ln(S*mag))
  e1 = exp(t) = S*mag;  e = exp(e1)  (softmax numerator, no max needed)
  f = exp(e1 - t) = e/(S*mag);  attnU = z * f   (= 8 * e * unit_phase)
  oh^T = sum_k wcombo^T @ attnU^T  ; normalized by 1/(8*sum_k e)
  out = oh @ Wout^T  (+bias, which is zero)
"""

import os
import sys
import numpy as np
import ml_dtypes

sys.path.insert(0, "/opt/trn_rl_repo")

from contextlib import ExitStack

import concourse.bass as bass
import concourse.tile as tile
from concourse import bacc, mybir, dve_ops
from concourse.bass_utils import run_bass_kernel_spmd
from concourse.dve_spec import Spec, Src0, Src1, sq, lower, _has_src1
from concourse.dve_uop import DveOpSpec


def _register_sqmag():
    """Custom DVE op: out = Src0^2 + Src1^2 (one instr instead of 2 ACT + 1 DVE)."""
    name = "TENSOR_SQMAG_ANT"
    if name in dve_ops._SUB_OPCODE_FOR_NAME:
        return next(o for o in dve_ops.OPS if o.name == name)
    spec = Spec(
        body=sq(Src0) + sq(Src1),
        reference=lambda in0, in1, s0, s1, imm2:
            (in0.astype(np.float32) ** 2 + in1.astype(np.float32) ** 2),
    )
    opcode = max(dve_ops._SUB_OPCODE_FOR_NAME.values()) + 1
    dve_ops._SUB_OPCODE_FOR_NAME[name] = opcode
    shas = {}
    for ver in ("v3", "v4"):
        uops = lower(spec, ver=ver)
        shas[ver] = DveOpSpec(name=name, opcode=opcode, uops=uops,
                              rd1_en=_has_src1(spec)).sha(ver)
    op = dve_ops.DveOp(name, spec, subdim=False, uops_sha=shas)
    dve_ops.OPS.append(op)
    dve_ops.CUSTOM_DVE_SPECS[name] = spec
    return op


SQMAG = _register_sqmag()


def _patch_act_tables():
    """Force exp/ln to resolve to the combined natural_log_exp_and_others set
    so the per-tile Ln->Exp alternation doesn't reload ACT tables (~2.7us each)."""
    import concourse.bacc as _bacc
    if getattr(_bacc, "_act_tables_patched", False):
        return
    orig = _bacc.get_activation_tables
    AFT = mybir.ActivationFunctionType

    def patched(arch):
        t = {k: set(v) for k, v in orig(arch).items()}
        for name, fns in t.items():
            if name != "natural_log_exp_and_others":
                fns.discard(AFT.Exp)
                fns.discard(AFT.Ln)
        return t

    _bacc.get_activation_tables = patched
    _bacc._act_tables_patched = True


_patch_act_tables()

B, N, D, H, DH = 4, 1024, 1024, 16, 64
E = H * DH          # 1024
NQ = 512            # queries per core
KT = 8              # key tiles of 128
DT = 8              # d (contraction) tiles of 128
ET = 8              # e tiles of 128 (2 heads each)
SCALE = DH ** -0.5  # 0.125
LN_S = float(np.log(SCALE))
EPS = 1e-20

FP32 = mybir.dt.float32
BF16 = mybir.dt.bfloat16
AF = mybir.ActivationFunctionType
ALU = mybir.AluOpType

_CACHE = {}


def build_graph():
    nc = bacc.Bacc("TRN2", target_bir_lowering=False, debug=False,
                   enable_asserts=False, num_devices=8)

    xTr_d = nc.declare_dram_parameter("xTr", [D, N], BF16, isOutput=False)
    xTi_d = nc.declare_dram_parameter("xTi", [D, N], BF16, isOutput=False)
    wqr_d = nc.declare_dram_parameter("wqTr", [D, E], BF16, isOutput=False)   # Wqkv_re.T
    wqi_d = nc.declare_dram_parameter("wqTi", [D, E], BF16, isOutput=False)   # Wqkv_im.T
    wqin_d = nc.declare_dram_parameter("wqTin", [D, E], BF16, isOutput=False)  # -Wqkv_im.T
    wo_d = nc.declare_dram_parameter("woS", [2 * E, D], BF16, isOutput=False)  # [Wout_re.T; Wout_im.T]
    our_d = nc.declare_dram_parameter("out_re", [NQ, D], FP32, isOutput=True)
    oui_d = nc.declare_dram_parameter("out_im", [NQ, D], FP32, isOutput=True)

    with tile.TileContext(nc) as tc, ExitStack() as ctx:
        const_pool = ctx.enter_context(tc.tile_pool(name="const", bufs=1))
        xpool = ctx.enter_context(tc.tile_pool(name="x", bufs=1))
        wqpool = ctx.enter_context(tc.tile_pool(name="wq", bufs=1))
        apool = ctx.enter_context(tc.tile_pool(name="A", bufs=1))
        bpool = ctx.enter_context(tc.tile_pool(name="B", bufs=2))
        wcpool = ctx.enter_context(tc.tile_pool(name="wc", bufs=2))
        ohpool = ctx.enter_context(tc.tile_pool(name="oh", bufs=1))
        wopool = ctx.enter_context(tc.tile_pool(name="wo", bufs=1))
        epool = ctx.enter_context(tc.tile_pool(name="elem", bufs=2))
        spool = ctx.enter_context(tc.tile_pool(name="sm", bufs=2))
        opool = ctx.enter_context(tc.tile_pool(name="ostage", bufs=1))
        psd = ctx.enter_context(tc.tile_pool(name="psd", bufs=3, space="PSUM"))
        psoh = ctx.enter_context(tc.tile_pool(name="psoh", bufs=1, space="PSUM"))
        psS = ctx.enter_context(tc.tile_pool(name="psS", bufs=1, space="PSUM"))

        # ---- constants ----
        ones_bf = const_pool.tile([128, 1], BF16, tag="ones")
        nc.gpsimd.memset(ones_bf[:], 1.0)
        eps_t = const_pool.tile([128, 1], FP32, tag="eps")
        nc.gpsimd.memset(eps_t[:], EPS)
        lnS_t = const_pool.tile([128, 1], FP32, tag="lnS")
        nc.gpsimd.memset(lnS_t[:], LN_S)
        ones8 = const_pool.tile([1, 128], FP32, tag="ones8")
        nc.gpsimd.memset(ones8[:], SCALE)

        # ---- resident loads ----
        # x^T as [128, DT, N] (partition = d within tile)
        xr = xpool.tile([128, DT, N], BF16, tag="xr")
        xi = xpool.tile([128, DT, N], BF16, tag="xi")
        nc.sync.dma_start(out=xr[:], in_=xTr_d.ap().rearrange("(t p) n -> p t n", p=128))
        nc.sync.dma_start(out=xi[:], in_=xTi_d.ap().rearrange("(t p) n -> p t n", p=128))
        # A stacks: per head [128 (wr 0:64 | wi 64:128), N]
        A = [apool.tile([128, N], BF16, tag=f"A{h}", name=f"A{h}") for h in range(H)]

        # ---- stage 1: w^T = Wqkv~ @ x^T, evicted to per-head stacks ----
        for et in range(ET):
            wslab_r = wqpool.tile([128, DT, 128], BF16, tag="wslab_r")
            wslab_i = wqpool.tile([128, DT, 128], BF16, tag="wslab_i")
            wslab_in = wqpool.tile([128, DT, 128], BF16, tag="wslab_in")
            esl = slice(et * 128, (et + 1) * 128)
            nc.sync.dma_start(out=wslab_r[:], in_=wqr_d.ap()[:, esl].rearrange("(t p) n -> p t n", p=128))
            nc.sync.dma_start(out=wslab_i[:], in_=wqi_d.ap()[:, esl].rearrange("(t p) n -> p t n", p=128))
            nc.sync.dma_start(out=wslab_in[:], in_=wqin_d.ap()[:, esl].rearrange("(t p) n -> p t n", p=128))
            ps_re = psd.tile([128, 2, 512], FP32, tag="dots")
            ps_im = psd.tile([128, 2, 512], FP32, tag="dots")
            for nch in range(2):
                nsl = slice(nch * 512, (nch + 1) * 512)
                for dt_ in range(DT):
                    first, last = dt_ == 0, dt_ == DT - 1
                    # w_re += Wr^T x_r ; w_re += (-Wi^T) x_i
                    nc.tensor.matmul(ps_re[:, nch, :], wslab_r[:, dt_, :], xr[:, dt_, nsl],
                                     start=first, stop=False)
                    nc.tensor.matmul(ps_re[:, nch, :], wslab_in[:, dt_, :], xi[:, dt_, nsl],
                                     start=False, stop=last)
                    # w_im += Wi^T x_r ; w_im += Wr^T x_i
                    nc.tensor.matmul(ps_im[:, nch, :], wslab_i[:, dt_, :], xr[:, dt_, nsl],
                                     start=first, stop=False)
                    nc.tensor.matmul(ps_im[:, nch, :], wslab_r[:, dt_, :], xi[:, dt_, nsl],
                                     start=False, stop=last)
            # evict into head stacks (cast to bf16)
            h0, h1 = 2 * et, 2 * et + 1
            Ar = A[h0].rearrange("p (c n) -> p c n", c=2)
            Ai = A[h1].rearrange("p (c n) -> p c n", c=2)
            nc.scalar.copy(Ar[0:64, :, :], ps_re[0:64, :, :])
            nc.scalar.copy(Ai[0:64, :, :], ps_re[64:128, :, :])
            nc.scalar.copy(Ar[64:128, :, :], ps_im[0:64, :, :])
            nc.scalar.copy(Ai[64:128, :, :], ps_im[64:128, :, :])

        # ---- per-head attention ----
        # oh^T stacks for stage 4: [128, ET, NQ] bf16
        ohr = ohpool.tile([128, ET, NQ], BF16, tag="ohr")
        ohi = ohpool.tile([128, ET, NQ], BF16, tag="ohi")
        ohin = ohpool.tile([128, ET, NQ], BF16, tag="ohin")

        for h in range(H):
            Ah = A[h]
            # B_h = [-wi; wr]
            Bh = bpool.tile([128, N], BF16, tag="B")
            nc.vector.tensor_scalar_mul(Bh[0:64, :], Ah[64:128, :], -1.0)
            nc.vector.tensor_copy(Bh[64:128, :], Ah[0:64, :])

            # wcombo1 = w_nat [k, (wr|wi)]; wcombo2 = [-wi|wr] per k-tile
            wc1 = wcpool.tile([128, KT, 128], BF16, tag="wc1")
            wc2 = wcpool.tile([128, KT, 128], BF16, tag="wc2")
            nc.sync.dma_start(wc1[:], Ah[:], transpose=True)
            nc.sync.dma_start(wc2[:], Bh[:], transpose=True)

            ps_oh = psoh.tile([128, NQ], FP32, tag="oh")
            ps_s = psS.tile([1, NQ], FP32, tag="S")

            for kp in range(KT // 2):
                zre = psd.tile([128, 2, NQ], FP32, tag="dots")
                zim = psd.tile([128, 2, NQ], FP32, tag="dots")
                # dots^T[k, q]: re = [wr;wi]_k . [wr;wi]_q ; im = [-wi;wr]_k . [wr;wi]_q
                for i in range(2):
                    ksl = slice((2 * kp + i) * 128, (2 * kp + i + 1) * 128)
                    nc.tensor.matmul(zre[:, i, :], Ah[:, ksl], Ah[:, 0:NQ],
                                     start=True, stop=True)
                    nc.tensor.matmul(zim[:, i, :], Bh[:, ksl], Ah[:, 0:NQ],
                                     start=True, stop=True)

                # evict both dots tensors as bf16: PSUM recycles fast and the
                # tail multiplies run in DVE 2x mode
                zreS = epool.tile([128, 2, NQ], BF16, tag="zreS", bufs=3)
                nc.scalar.copy(zreS[:], zre[:])
                zimS = epool.tile([128, 2, NQ], BF16, tag="zimS", bufs=3)
                if kp % 2 == 0:
                    nc.scalar.copy(zimS[:], zim[:])
                else:
                    nc.vector.tensor_copy(zimS[:], zim[:])
                m2 = epool.tile([128, 2, NQ], FP32, tag="m2", bufs=3)
                nc.vector._custom_dve(SQMAG, out=m2[:], in0=zreS[:], in1=zimS[:])
                ll = epool.tile([128, 2, NQ], FP32, tag="ll", bufs=3)
                nc.scalar.activation(ll[:], m2[:], AF.Ln, bias=eps_t[:])
                # e1 = S*mag = exp(0.5*ln(m2) + ln(S)) -- affine fused into the ACT op
                e1 = epool.tile([128, 2, NQ], FP32, tag="e1", bufs=2)
                nc.scalar.activation(e1[:], ll[:], AF.Exp, scale=0.5, bias=lnS_t[:])
                ee = epool.tile([128, 2, NQ], BF16, tag="ee", bufs=4)
                nc.scalar.activation(ee[:], e1[:], AF.Exp)
                rm = epool.tile([128, 2, NQ], FP32, tag="rm", bufs=2)
                nc.vector.reciprocal_approx_fast(out=rm[:], in_=e1[:])
                ff = epool.tile([128, 2, NQ], BF16, tag="ff", bufs=3)
                nc.vector.tensor_mul(ff[:], ee[:], rm[:])
                are = epool.tile([128, 2, NQ], BF16, tag="are", bufs=2)
                aim = epool.tile([128, 2, NQ], BF16, tag="aim", bufs=2)
                nc.vector.tensor_mul(are[:], zreS[:], ff[:])
                nc.vector.tensor_mul(aim[:], zimS[:], ff[:])

                for i in range(2):
                    kt = 2 * kp + i
                    first, last = kt == 0, kt == KT - 1
                    nc.tensor.matmul(ps_oh[:], wc1[:, kt, :], are[:, i, :],
                                     start=first, stop=False)
                    nc.tensor.matmul(ps_oh[:], wc2[:, kt, :], aim[:, i, :],
                                     start=False, stop=last)
                    nc.tensor.matmul(ps_s[:], ones_bf[:], ee[:, i, :],
                                     start=first, stop=last)

            # evict raw oh^T immediately (releases psoh for the next head)
            ohraw = spool.tile([128, NQ], FP32, tag="ohraw", bufs=2)
            nc.scalar.copy(ohraw[:], ps_oh[:])
            # rs = 1/S (psS released); broadcast SCALE/S via ones-matmul into PSUM
            rs = spool.tile([1, NQ], FP32, tag="rs", bufs=1)
            nc.vector.reciprocal_approx_fast(out=rs[:], in_=ps_s[:])
            bb = psd.tile([128, 2, NQ], FP32, tag="dots")
            nc.tensor.matmul(bb[:, 0, :], ones8[:], rs[:], start=True, stop=True)

            et2, half = h // 2, (h % 2) * 64
            hs = slice(half, half + 64)
            nc.vector.tensor_mul(ohr[hs, et2, :], ohraw[0:64, :], bb[0:64, 0, :])
            nc.vector.tensor_mul(ohi[hs, et2, :], ohraw[64:128, :], bb[64:128, 0, :])
            nc.vector.tensor_scalar_mul(ohin[hs, et2, :], ohi[hs, et2, :], -1.0)

        # ---- stage 4: out = oh @ Wout^T ----
        for dc in range(2):
            dsl = slice(dc * 512, (dc + 1) * 512)
            wos = wopool.tile([128, 16, 512], BF16, tag="wos")
            nc.sync.dma_start(out=wos[:], in_=wo_d.ap()[:, dsl].rearrange("(t p) n -> p t n", p=128))
            for qt in range(4):
                qsl = slice(qt * 128, (qt + 1) * 128)
                po = psd.tile([128, 2, 512], FP32, tag="dots")
                for et in range(ET):
                    first = et == 0
                    nc.tensor.matmul(po[:, 0, :], ohr[:, et, qsl], wos[:, et, :],
                                     start=first, stop=False)
                    nc.tensor.matmul(po[:, 0, :], ohin[:, et, qsl], wos[:, ET + et, :],
                                     start=False, stop=(et == ET - 1))
                    nc.tensor.matmul(po[:, 1, :], ohi[:, et, qsl], wos[:, et, :],
                                     start=first, stop=False)
                    nc.tensor.matmul(po[:, 1, :], ohr[:, et, qsl], wos[:, ET + et, :],
                                     start=False, stop=(et == ET - 1))
                o_st = opool.tile([128, 2, 512], FP32, tag="ost", bufs=1)
                nc.scalar.copy(o_st[:], po[:])
                nc.sync.dma_start(out=our_d.ap()[qsl, dsl], in_=o_st[:, 0, :])
                nc.sync.dma_start(out=oui_d.ap()[qsl, dsl], in_=o_st[:, 1, :])

    nc.compile()
    return nc


def _to_bf16(a):
    return np.asarray(a, dtype=np.float32).astype(ml_dtypes.bfloat16)


def make_in_maps(x_re, x_im, wqkv_re, wqkv_im, wout_re, wout_im, bout_re, bout_im):
    x_re = np.asarray(x_re, np.float32)
    x_im = np.asarray(x_im, np.float32)
    wq_r = _to_bf16(np.asarray(wqkv_re, np.float32).T)
    wq_i = _to_bf16(np.asarray(wqkv_im, np.float32).T)
    wq_in = _to_bf16(-np.asarray(wqkv_im, np.float32).T)
    wo_s = _to_bf16(np.concatenate([np.asarray(wout_re, np.float32).T,
                                    np.asarray(wout_im, np.float32).T], axis=0))

    in_maps = []
    for c in range(8):
        b, qh = c // 2, c % 2
        xtr = x_re[b].T
        xti = x_im[b].T
        if qh == 1:
            xtr = np.concatenate([xtr[:, NQ:], xtr[:, :NQ]], axis=1)
            xti = np.concatenate([xti[:, NQ:], xti[:, :NQ]], axis=1)
        in_maps.append({
            "xTr": _to_bf16(np.ascontiguousarray(xtr)),
            "xTi": _to_bf16(np.ascontiguousarray(xti)),
            "wqTr": wq_r, "wqTi": wq_i, "wqTin": wq_in, "woS": wo_s,
        })
    return in_maps


def assemble_output(res, bout_re, bout_im):
    out = np.zeros((B, N, D), np.complex64)
    for c in range(8):
        b, qh = c // 2, c % 2
        rows = slice(0, NQ) if qh == 0 else slice(NQ, N)
        out[b, rows, :] = res[c]["out_re"] + 1j * res[c]["out_im"]

    # bout is zero in this problem; add anyway for faithfulness
    out += (np.asarray(bout_re, np.float32) + 1j * np.asarray(bout_im, np.float32))[None, None, :]
    return out


def kernel(**inputs):
    if "nc" not in _CACHE:
        _CACHE["nc"] = build_graph()
    nc = _CACHE["nc"]
    in_maps = make_in_maps(**inputs)
    res = run_bass_kernel_spmd(nc, in_maps, core_ids=list(range(8))).results
    return assemble_output(res, inputs["bout_re"], inputs["bout_im"])


# revision 46
# speedup vs baseline: 1.3667x; 1.0528x over previous
"""Complex-valued attention (magnitude-softmax with phase reconstruction) on 8 TRN2 cores.

Sharding: core c -> (batch b = c//2, query-half qh = c%2). No collectives:
each core computes a disjoint [512, 1024] slice of the output. The query
half is selected by permuting the token axis of x^T host-side so that each
core's queries are always columns 0:512 of its shard (SPMD: one graph).

Math (per head h, scale S = 1/8):
  w = x @ Wqkv^T                         (bf16 matmuls, fp32 accum)
  z = dots^T[k, q] = <w_k-ish>           computed TRANSPOSED via stacked
      re/im contraction so softmax ends up on the partition axis
  m2 = z_re^2 + z_im^2; l = ln(m2+eps); t = 0.5*l + ln(S)  (t = ln(S*mag))
  e1 = exp(t) = S*mag;  e = exp(e1)  (softmax numerator, no max needed)
  f = exp(e1 - t) = e/(S*mag);  attnU = z * f   (= 8 * e * unit_phase)
  oh^T = sum_k wcombo^T @ attnU^T  ; normalized by 1/(8*sum_k e)
  out = oh @ Wout^T  (+bias, which is zero)
"""

import os
import sys
import numpy as np
import ml_dtypes

sys.path.insert(0, "/opt/trn_rl_repo")

from contextlib import ExitStack

import concourse.bass as bass
import concourse.tile as tile
from concourse import bacc, mybir, dve_ops
from concourse.bass_utils import run_bass_kernel_spmd
from concourse.dve_spec import Spec, Src0, Src1, sq, lower, _has_src1
from concourse.dve_uop import DveOpSpec


def _register_sqmag():
    """Custom DVE op: out = Src0^2 + Src1^2 (one instr instead of 2 ACT + 1 DVE)."""
    name = "TENSOR_SQMAG_ANT"
    if name in dve_ops._SUB_OPCODE_FOR_NAME:
        return next(o for o in dve_ops.OPS if o.name == name)
    spec = Spec(
        body=sq(Src0) + sq(Src1),
        reference=lambda in0, in1, s0, s1, imm2:
            (in0.astype(np.float32) ** 2 + in1.astype(np.float32) ** 2),
    )
    opcode = max(dve_ops._SUB_OPCODE_FOR_NAME.values()) + 1
    dve_ops._SUB_OPCODE_FOR_NAME[name] = opcode
    shas = {}
    for ver in ("v3", "v4"):
        uops = lower(spec, ver=ver)
        shas[ver] = DveOpSpec(name=name, opcode=opcode, uops=uops,
                              rd1_en=_has_src1(spec)).sha(ver)
    op = dve_ops.DveOp(name, spec, subdim=False, uops_sha=shas)
    dve_ops.OPS.append(op)
    dve_ops.CUSTOM_DVE_SPECS[name] = spec
    return op


SQMAG = _register_sqmag()


def _patch_act_tables():
    """Force exp/ln to resolve to the combined natural_log_exp_and_others set
    so the per-tile Ln->Exp alternation doesn't reload ACT tables (~2.7us each)."""
    import concourse.bacc as _bacc
    if getattr(_bacc, "_act_tables_patched", False):
        return
    orig = _bacc.get_activation_tables
    AFT = mybir.ActivationFunctionType

    def patched(arch):
        t = {k: set(v) for k, v in orig(arch).items()}
        for name, fns in t.items():
            if name != "natural_log_exp_and_others":
                fns.discard(AFT.Exp)
                fns.discard(AFT.Ln)
        return t

    _bacc.get_activation_tables = patched
    _bacc._act_tables_patched = True


_patch_act_tables()

B, N, D, H, DH = 4, 1024, 1024, 16, 64
E = H * DH          # 1024
NQ = 512            # queries per core
KT = 8              # key tiles of 128
DT = 8              # d (contraction) tiles of 128
ET = 8              # e tiles of 128 (2 heads each)
SCALE = DH ** -0.5  # 0.125
LN_S = float(np.log(SCALE))
EPS = 1e-20

FP32 = mybir.dt.float32
BF16 = mybir.dt.bfloat16
AF = mybir.ActivationFunctionType
ALU = mybir.AluOpType

_CACHE = {}


def build_graph():
    nc = bacc.Bacc("TRN2", target_bir_lowering=False, debug=False,
                   enable_asserts=False, num_devices=8)

    xTr_d = nc.declare_dram_parameter("xTr", [D, N], BF16, isOutput=False)
    xTi_d = nc.declare_dram_parameter("xTi", [D, N], BF16, isOutput=False)
    wqr_d = nc.declare_dram_parameter("wqTr", [D, E], BF16, isOutput=False)   # Wqkv_re.T
    wqi_d = nc.declare_dram_parameter("wqTi", [D, E], BF16, isOutput=False)   # Wqkv_im.T
    wqin_d = nc.declare_dram_parameter("wqTin", [D, E], BF16, isOutput=False)  # -Wqkv_im.T
    wo_d = nc.declare_dram_parameter("woS", [2 * E, D], BF16, isOutput=False)  # [Wout_re.T; Wout_im.T]
    our_d = nc.declare_dram_parameter("out_re", [NQ, D], FP32, isOutput=True)
    oui_d = nc.declare_dram_parameter("out_im", [NQ, D], FP32, isOutput=True)

    with tile.TileContext(nc) as tc, ExitStack() as ctx:
        const_pool = ctx.enter_context(tc.tile_pool(name="const", bufs=1))
        xpool = ctx.enter_context(tc.tile_pool(name="x", bufs=1))
        wqpool = ctx.enter_context(tc.tile_pool(name="wq", bufs=1))
        apool = ctx.enter_context(tc.tile_pool(name="A", bufs=1))
        bpool = ctx.enter_context(tc.tile_pool(name="B", bufs=2))
        wcpool = ctx.enter_context(tc.tile_pool(name="wc", bufs=2))
        ohpool = ctx.enter_context(tc.tile_pool(name="oh", bufs=1))
        wopool = ctx.enter_context(tc.tile_pool(name="wo", bufs=1))
        epool = ctx.enter_context(tc.tile_pool(name="elem", bufs=2))
        spool = ctx.enter_context(tc.tile_pool(name="sm", bufs=2))
        opool = ctx.enter_context(tc.tile_pool(name="ostage", bufs=1))
        psd = ctx.enter_context(tc.tile_pool(name="psd", bufs=3, space="PSUM"))
        psoh = ctx.enter_context(tc.tile_pool(name="psoh", bufs=1, space="PSUM"))
        psS = ctx.enter_context(tc.tile_pool(name="psS", bufs=1, space="PSUM"))

        # ---- constants ----
        ones_bf = const_pool.tile([128, 1], BF16, tag="ones")
        nc.gpsimd.memset(ones_bf[:], 1.0)
        eps_t = const_pool.tile([128, 1], FP32, tag="eps")
        nc.gpsimd.memset(eps_t[:], EPS)
        lnS_t = const_pool.tile([128, 1], FP32, tag="lnS")
        nc.gpsimd.memset(lnS_t[:], LN_S)
        ones8 = const_pool.tile([1, 128], FP32, tag="ones8")
        nc.gpsimd.memset(ones8[:], SCALE)

        # ---- resident loads ----
        # x^T as [128, DT, N] (partition = d within tile)
        xr = xpool.tile([128, DT, N], BF16, tag="xr")
        xi = xpool.tile([128, DT, N], BF16, tag="xi")
        nc.sync.dma_start(out=xr[:], in_=xTr_d.ap().rearrange("(t p) n -> p t n", p=128))
        nc.sync.dma_start(out=xi[:], in_=xTi_d.ap().rearrange("(t p) n -> p t n", p=128))
        # A stacks: per head [128 (wr 0:64 | wi 64:128), N]
        A = [apool.tile([128, N], BF16, tag=f"A{h}", name=f"A{h}") for h in range(H)]

        # ---- stage 1: w^T = Wqkv~ @ x^T, evicted to per-head stacks ----
        for et in range(ET):
            wslab_r = wqpool.tile([128, DT, 128], BF16, tag="wslab_r")
            wslab_i = wqpool.tile([128, DT, 128], BF16, tag="wslab_i")
            wslab_in = wqpool.tile([128, DT, 128], BF16, tag="wslab_in")
            esl = slice(et * 128, (et + 1) * 128)
            nc.sync.dma_start(out=wslab_r[:], in_=wqr_d.ap()[:, esl].rearrange("(t p) n -> p t n", p=128))
            nc.sync.dma_start(out=wslab_i[:], in_=wqi_d.ap()[:, esl].rearrange("(t p) n -> p t n", p=128))
            nc.sync.dma_start(out=wslab_in[:], in_=wqin_d.ap()[:, esl].rearrange("(t p) n -> p t n", p=128))
            ps_re = psd.tile([128, 2, 512], FP32, tag="dots")
            ps_im = psd.tile([128, 2, 512], FP32, tag="dots")
            for nch in range(2):
                nsl = slice(nch * 512, (nch + 1) * 512)
                for dt_ in range(DT):
                    first, last = dt_ == 0, dt_ == DT - 1
                    # w_re += Wr^T x_r ; w_re += (-Wi^T) x_i
                    nc.tensor.matmul(ps_re[:, nch, :], wslab_r[:, dt_, :], xr[:, dt_, nsl],
                                     start=first, stop=False)
                    nc.tensor.matmul(ps_re[:, nch, :], wslab_in[:, dt_, :], xi[:, dt_, nsl],
                                     start=False, stop=last)
                    # w_im += Wi^T x_r ; w_im += Wr^T x_i
                    nc.tensor.matmul(ps_im[:, nch, :], wslab_i[:, dt_, :], xr[:, dt_, nsl],
                                     start=first, stop=False)
                    nc.tensor.matmul(ps_im[:, nch, :], wslab_r[:, dt_, :], xi[:, dt_, nsl],
                                     start=False, stop=last)
            # evict into head stacks (cast to bf16)
            h0, h1 = 2 * et, 2 * et + 1
            Ar = A[h0].rearrange("p (c n) -> p c n", c=2)
            Ai = A[h1].rearrange("p (c n) -> p c n", c=2)
            nc.scalar.copy(Ar[0:64, :, :], ps_re[0:64, :, :])
            nc.scalar.copy(Ai[0:64, :, :], ps_re[64:128, :, :])
            nc.scalar.copy(Ar[64:128, :, :], ps_im[0:64, :, :])
            nc.scalar.copy(Ai[64:128, :, :], ps_im[64:128, :, :])

        # ---- per-head attention ----
        # oh^T stacks for stage 4: [128, ET, NQ] bf16
        ohr = ohpool.tile([128, ET, NQ], BF16, tag="ohr")
        ohi = ohpool.tile([128, ET, NQ], BF16, tag="ohi")
        ohin = ohpool.tile([128, ET, NQ], BF16, tag="ohin")

        for h in range(H):
            Ah = A[h]
            # B_h = [-wi; wr]
            Bh = bpool.tile([128, N], BF16, tag="B")
            nc.vector.tensor_scalar_mul(Bh[0:64, :], Ah[64:128, :], -1.0)
            nc.vector.tensor_copy(Bh[64:128, :], Ah[0:64, :])

            # wcombo1 = w_nat [k, (wr|wi)]; wcombo2 = [-wi|wr] per k-tile
            wc1 = wcpool.tile([128, KT, 128], BF16, tag="wc1")
            wc2 = wcpool.tile([128, KT, 128], BF16, tag="wc2")
            nc.sync.dma_start(wc1[:], Ah[:], transpose=True)
            nc.sync.dma_start(wc2[:], Bh[:], transpose=True)

            ps_oh = psoh.tile([128, NQ], FP32, tag="oh")
            ps_s = psS.tile([1, NQ], FP32, tag="S")

            for kp in range(KT // 2):
                zre = psd.tile([128, 2, NQ], FP32, tag="dots")
                zim = psd.tile([128, 2, NQ], FP32, tag="dots")
                # dots^T[k, q]: re = [wr;wi]_k . [wr;wi]_q ; im = [-wi;wr]_k . [wr;wi]_q
                for i in range(2):
                    ksl = slice((2 * kp + i) * 128, (2 * kp + i + 1) * 128)
                    nc.tensor.matmul(zre[:, i, :], Ah[:, ksl], Ah[:, 0:NQ],
                                     start=True, stop=True)
                    nc.tensor.matmul(zim[:, i, :], Bh[:, ksl], Ah[:, 0:NQ],
                                     start=True, stop=True)

                # evict both dots tensors as bf16: PSUM recycles fast and the
                # tail multiplies run in DVE 2x mode
                zreS = epool.tile([128, 2, NQ], BF16, tag="zreS", bufs=3)
                nc.scalar.copy(zreS[:], zre[:])
                zimS = epool.tile([128, 2, NQ], BF16, tag="zimS", bufs=3)
                nc.scalar.copy(zimS[:], zim[:])
                m2 = epool.tile([128, 2, NQ], FP32, tag="m2", bufs=3)
                nc.vector._custom_dve(SQMAG, out=m2[:], in0=zreS[:], in1=zimS[:])
                ll = epool.tile([128, 2, NQ], FP32, tag="ll", bufs=3)
                nc.scalar.activation(ll[:], m2[:], AF.Ln, bias=eps_t[:])
                # e1 = S*mag = exp(0.5*ln(m2) + ln(S)) -- affine fused into the ACT op
                e1 = epool.tile([128, 2, NQ], FP32, tag="e1", bufs=2)
                nc.scalar.activation(e1[:], ll[:], AF.Exp, scale=0.5, bias=lnS_t[:])
                ee = epool.tile([128, 2, NQ], BF16, tag="ee", bufs=4)
                nc.scalar.activation(ee[:], e1[:], AF.Exp)
                rm = epool.tile([128, 2, NQ], FP32, tag="rm", bufs=2)
                nc.vector.reciprocal_approx_fast(out=rm[:], in_=e1[:])
                ff = epool.tile([128, 2, NQ], BF16, tag="ff", bufs=3)
                nc.vector.tensor_mul(ff[:], ee[:], rm[:])
                are = epool.tile([128, 2, NQ], BF16, tag="are", bufs=2)
                aim = epool.tile([128, 2, NQ], BF16, tag="aim", bufs=2)
                nc.vector.tensor_mul(are[:], zreS[:], ff[:])
                nc.vector.tensor_mul(aim[:], zimS[:], ff[:])

                for i in range(2):
                    kt = 2 * kp + i
                    first, last = kt == 0, kt == KT - 1
                    nc.tensor.matmul(ps_oh[:], wc1[:, kt, :], are[:, i, :],
                                     start=first, stop=False)
                    nc.tensor.matmul(ps_oh[:], wc2[:, kt, :], aim[:, i, :],
                                     start=False, stop=last)
                    nc.tensor.matmul(ps_s[:], ones_bf[:], ee[:, i, :],
                                     start=first, stop=last)

            # evict raw oh^T immediately (releases psoh for the next head)
            ohraw = spool.tile([128, NQ], FP32, tag="ohraw", bufs=2)
            nc.scalar.copy(ohraw[:], ps_oh[:])
            # rs = 1/S (psS released); broadcast SCALE/S via ones-matmul into PSUM
            rs = spool.tile([1, NQ], FP32, tag="rs", bufs=1)
            nc.vector.reciprocal_approx_fast(out=rs[:], in_=ps_s[:])
            bb = psd.tile([128, 2, NQ], FP32, tag="dots")
            nc.tensor.matmul(bb[:, 0, :], ones8[:], rs[:], start=True, stop=True)

            et2, half = h // 2, (h % 2) * 64
            hs = slice(half, half + 64)
            nc.vector.tensor_mul(ohr[hs, et2, :], ohraw[0:64, :], bb[0:64, 0, :])
            nc.vector.tensor_mul(ohi[hs, et2, :], ohraw[64:128, :], bb[64:128, 0, :])
            nc.vector.tensor_scalar_mul(ohin[hs, et2, :], ohi[hs, et2, :], -1.0)

        # ---- stage 4: out = oh @ Wout^T ----
        for dc in range(2):
            dsl = slice(dc * 512, (dc + 1) * 512)
            wos = wopool.tile([128, 16, 512], BF16, tag="wos")
            nc.sync.dma_start(out=wos[:], in_=wo_d.ap()[:, dsl].rearrange("(t p) n -> p t n", p=128))
            for qt in range(4):
                qsl = slice(qt * 128, (qt + 1) * 128)
                po = psd.tile([128, 2, 512], FP32, tag="dots")
                for et in range(ET):
                    first = et == 0
                    nc.tensor.matmul(po[:, 0, :], ohr[:, et, qsl], wos[:, et, :],
                                     start=first, stop=False)
                    nc.tensor.matmul(po[:, 0, :], ohin[:, et, qsl], wos[:, ET + et, :],
                                     start=False, stop=(et == ET - 1))
                    nc.tensor.matmul(po[:, 1, :], ohi[:, et, qsl], wos[:, et, :],
                                     start=first, stop=False)
                    nc.tensor.matmul(po[:, 1, :], ohr[:, et, qsl], wos[:, ET + et, :],
                                     start=False, stop=(et == ET - 1))
                o_st = opool.tile([128, 2, 512], FP32, tag="ost", bufs=1)
                nc.scalar.copy(o_st[:], po[:])
                nc.sync.dma_start(out=our_d.ap()[qsl, dsl], in_=o_st[:, 0, :])
                nc.sync.dma_start(out=oui_d.ap()[qsl, dsl], in_=o_st[:, 1, :])

    nc.compile()
    return nc


def _to_bf16(a):
    return np.asarray(a, dtype=np.float32).astype(ml_dtypes.bfloat16)


def make_in_maps(x_re, x_im, wqkv_re, wqkv_im, wout_re, wout_im, bout_re, bout_im):
    x_re = np.asarray(x_re, np.float32)
    x_im = np.asarray(x_im, np.float32)
    wq_r = _to_bf16(np.asarray(wqkv_re, np.float32).T)
    wq_i = _to_bf16(np.asarray(wqkv_im, np.float32).T)
    wq_in = _to_bf16(-np.asarray(wqkv_im, np.float32).T)
    wo_s = _to_bf16(np.concatenate([np.asarray(wout_re, np.float32).T,
                                    np.asarray(wout_im, np.float32).T], axis=0))

    in_maps = []
    for c in range(8):
        b, qh = c // 2, c % 2
        xtr = x_re[b].T
        xti = x_im[b].T
        if qh == 1:
            xtr = np.concatenate([xtr[:, NQ:], xtr[:, :NQ]], axis=1)
            xti = np.concatenate([xti[:, NQ:], xti[:, :NQ]], axis=1)
        in_maps.append({
            "xTr": _to_bf16(np.ascontiguousarray(xtr)),
            "xTi": _to_bf16(np.ascontiguousarray(xti)),
            "wqTr": wq_r, "wqTi": wq_i, "wqTin": wq_in, "woS": wo_s,
        })
    return in_maps


def assemble_output(res, bout_re, bout_im):
    out = np.zeros((B, N, D), np.complex64)
    for c in range(8):
        b, qh = c // 2, c % 2
        rows = slice(0, NQ) if qh == 0 else slice(NQ, N)
        out[b, rows, :] = res[c]["out_re"] + 1j * res[c]["out_im"]

    # bout is zero in this problem; add anyway for faithfulness
    out += (np.asarray(bout_re, np.float32) + 1j * np.asarray(bout_im, np.float32))[None, None, :]
    return out


def kernel(**inputs):
    if "nc" not in _CACHE:
        _CACHE["nc"] = build_graph()
    nc = _CACHE["nc"]
    in_maps = make_in_maps(**inputs)
    res = run_bass_kernel_spmd(nc, in_maps, core_ids=list(range(8))).results
    return assemble_output(res, inputs["bout_re"], inputs["bout_im"])
